# revision 13
# baseline (speedup 1.0000x reference)
"""Multi-head attention (B=2, N=2048, D=1024, H=16, dh=64) on 8 TRN2 cores.

Sharding: (batch x head-group) -- core c handles batch c//4 and heads
[4*(c%4), 4*(c%4)+4) (256 local dims = 2 head-pairs). Host sums 4
partials per batch and adds bo.

Per-core pipeline (v2): the PE array is time-packed via tile_position
so the attention matmuls use the full 128x128 array:
  - scores: heads 2t/2t+1 run CONCURRENTLY as 64x128 row-tiles
    (contraction dh=64; lhsT/rhs at partitions 0-63 vs 64-127).
  - ctx: the same pair runs concurrently as 128x64 col-tiles
    (lhsT = V_h [128,64]; outputs at psum partitions 0-63 / 64-127).
  - softmax denominator Z: M=1 ones-matmuls at col positions
    (0,0),(0,32),(0,64),(0,96) -- 4 concurrent streams covering
    (head A/B) x (even/odd kt), accumulated over kt in ONE psum bank.
    A K=4 selector matmul later broadcasts (Z_even+Z_odd) per head
    across 64 partitions.
  - exp runs on ACT as [128,1024] insts reading a 4-slot psum quad
    (2 slots per kt: A|B), ring-recycled; ACT is the steady-state
    bottleneck at ~1147ns/kt.
Work is organized as 8 phases (pair-major, q-half 512): pair p, then
q-halves 0..3. V projection (x^T stationary) rides phase 0-1; Q/K
projections for pair 1 ride phases 1-3 as deferred PE units; the
output projection (per q-half, both pairs) rides phases 4-7.
"""

import numpy as np
import ml_dtypes
from contextlib import ExitStack

import concourse.bass as bass
import concourse.tile as tile
from concourse import bacc, mybir
from concourse.bass import ts, ds
from concourse.bass_utils import run_bass_kernel_spmd

BF16 = mybir.dt.bfloat16
F32 = mybir.dt.float32

B = 2
N = 2048          # tokens per batch
D = 1024          # model dim
NCORES = 8
HLOC = 4          # heads per core
DLOC = 256        # local dims per core
DH = 64
NKT = N // 128    # 16 k-tiles
NDCH = D // 128   # 8 d-chunks
QW = 512          # q-half width
NQH = N // QW     # 4 q-halves
NSLOT = 40        # e-ring slots of [128, 512]


def _build_program():
    nc = bacc.Bacc("TRN2", target_bir_lowering=False, debug=False)

    xT = {}
    w = {}
    for t in ("q", "k", "v"):
        xT[t] = nc.dram_tensor(f"x{t}T", [D, N], BF16, kind="ExternalInput").ap()
        w[t] = nc.dram_tensor(f"w{t}", [D, DLOC], BF16, kind="ExternalInput").ap()
    bias = {}
    for t in ("q", "k"):
        bias[t] = nc.dram_tensor(f"b{t}", [DLOC, 1], F32, kind="ExternalInput").ap()
    bvd = nc.dram_tensor("bv", [1, DLOC], F32, kind="ExternalInput").ap()
    wo = nc.dram_tensor("wo", [DLOC, D], BF16, kind="ExternalInput").ap()
    outp = nc.dram_tensor("outp", [N, D], BF16, kind="ExternalOutput").ap()

    with ExitStack() as ctx:
        tc = ctx.enter_context(tile.TileContext(nc))

        const = ctx.enter_context(tc.tile_pool(name="const", bufs=1))
        xqkp = ctx.enter_context(tc.tile_pool(name="xqk", bufs=32))
        xvp = ctx.enter_context(tc.tile_pool(name="xv", bufs=8))
        qkp = ctx.enter_context(tc.tile_pool(name="qk", bufs=1))
        vaugp = ctx.enter_context(tc.tile_pool(name="vaug", bufs=1))
        eringp = ctx.enter_context(tc.tile_pool(name="ering", bufs=1))
        cxp = ctx.enter_context(tc.tile_pool(name="cxu", bufs=2))
        zsbp = ctx.enter_context(tc.tile_pool(name="zsb", bufs=2))
        recp = ctx.enter_context(tc.tile_pool(name="rec", bufs=2))
        stackp = ctx.enter_context(tc.tile_pool(name="stack", bufs=4))
        obp = ctx.enter_context(tc.tile_pool(name="ob", bufs=4))

        # PSUM: squad 4 banks (scores ring), pctx 1 (ctx accum), pz 1
        # (Z accum), pwo 1 (psv then wo), pfl 1 (proj chunks, bc, wo)
        psqp = ctx.enter_context(tc.tile_pool(name="psq", bufs=1, space="PSUM"))
        pctxp = ctx.enter_context(tc.tile_pool(name="pctx", bufs=1, space="PSUM"))
        pzp = ctx.enter_context(tc.tile_pool(name="pz", bufs=1, space="PSUM"))
        pwop = ctx.enter_context(tc.tile_pool(name="pwo", bufs=1, space="PSUM"))
        pflp = ctx.enter_context(tc.tile_pool(name="pfl", bufs=1, space="PSUM"))

        # ---- constants + x loads, ordered for just-in-time arrival ----
        w_sb = {}
        b_sb = {}
        xtiles = {}

        def load_x(t, hf):
            for c in range(NDCH):
                xt_ = xqkp.tile([128, 1024], BF16, tag="x", name="xt")
                nc.sync.dma_start(out=xt_, in_=xT[t][ts(c, 128), ts(hf, 1024)])
                xtiles[(t, c, hf)] = xt_

        for t in ("q", "k"):
            w_sb[t] = const.tile([128, NDCH, DLOC], BF16, tag=f"w{t}", name=f"w{t}sb")
            nc.sync.dma_start(out=w_sb[t], in_=w[t].rearrange("(c p) m -> p c m", p=128))
            b_sb[t] = const.tile([128, 2, 1], F32, tag=f"b{t}", name=f"b{t}sb")
            nc.sync.dma_start(out=b_sb[t],
                              in_=bias[t].rearrange("(t p) o -> p t o", p=128))
        load_x("k", 0)
        load_x("q", 0)
        load_x("k", 1)
        load_x("q", 1)
        w_sb["v"] = const.tile([128, NDCH, DLOC], BF16, tag="wv", name="wvsb")
        nc.sync.dma_start(out=w_sb["v"], in_=w["v"].rearrange("(c p) m -> p c m", p=128))
        for c in range(NDCH):
            xt_ = xvp.tile([128, N], BF16, tag="xv", name="xvt")
            nc.sync.dma_start(out=xt_, in_=xT["v"][ts(c, 128), :])
            xtiles[("v", c)] = xt_
        bvbc = const.tile([128, HLOC, DH], F32, tag="bvbc")
        seg = bvd[0, :]
        nc.sync.dma_start(
            out=bvbc,
            in_=bass.AP(tensor=seg.tensor, offset=seg.offset,
                        ap=[[0, 128]] + list(seg.ap)))
        wo_sb = const.tile([128, 2, D], BF16, tag="wo")
        nc.sync.dma_start(out=wo_sb, in_=wo.rearrange("(t p) d -> p t d", p=128))

        # warm the exp table load off the critical path
        warm = const.tile([128, 1], BF16, tag="warm")
        nc.scalar.activation(warm, b_sb["q"][:, 0, :], mybir.ActivationFunctionType.Exp)

        # ones column for Z matmuls; K=4 selector for the Z broadcast:
        # bc[m] = sum_j sel4[j, m] * zsb[j]; rows 0-63 <- z[0]+z[2] (head A
        # even+odd kt), rows 64-127 <- z[1]+z[3] (head B)
        ones_t = const.tile([128, 1], BF16, tag="ones")
        nc.vector.memset(ones_t, 1.0)
        sel128 = const.tile([128, 128], BF16, tag="sel128")
        nc.vector.memset(sel128, 0.0)
        nc.vector.memset(sel128[0:1, 0:64], 1.0)
        nc.vector.memset(sel128[64:65, 0:64], 1.0)
        nc.vector.memset(sel128[32:33, 64:128], 1.0)
        nc.vector.memset(sel128[96:97, 64:128], 1.0)

        # ---- PE warm-up burst during the DMA lead-in (HAM clock gate) ----
        wmt = const.tile([128, 512], BF16, tag="wmt")
        nc.vector.memset(wmt, 0.0)
        squad = psqp.tile([128, 4, 512], F32, tag="sq", name="squad")
        for i in range(10):
            nc.tensor.matmul(squad[:, 0, :], lhsT=wmt[:, 0:128], rhs=wmt,
                             start=(i == 0), stop=(i == 9))

        # ---- projections ----
        # Q^T/K^T [dl, tok] per pair (dl-tile), W stationary. One chunk =
        # one (t, pair, sh) with 8 accumulating c-matmuls into a [128,512]
        # psum + a bias-add evac. Prelude: K/Q pair0 sh0 on squad slots;
        # the rest ride phases as deferred PE units.
        qt_sb = qkp.tile([128, 2, N], BF16, tag="qt")
        kt_sb = qkp.tile([128, 2, N], BF16, tag="kt")
        dest = {"q": qt_sb, "k": kt_sb}

        def proj_chunk(t, pair, sh, ps):
            for c in range(NDCH):
                nc.tensor.matmul(
                    ps,
                    lhsT=w_sb[t][:, c, ds(pair * 128, 128)],
                    rhs=xtiles[(t, c, sh // 2)][:, ds((sh % 2) * 512, 512)],
                    start=(c == 0),
                    stop=(c == NDCH - 1),
                )
            nc.vector.tensor_scalar_add(
                dest[t][:, pair, ts(sh, 512)], ps, b_sb[t][:, pair, :])

        proj_chunk("k", 0, 0, squad[:, 0, :])
        proj_chunk("q", 0, 0, squad[:, 1, :])
        proj_chunk("k", 0, 1, squad[:, 2, :])
        proj_chunk("q", 0, 1, squad[:, 3, :])

        # deferred PE units (each ~2us of PE work), ordered by need:
        # k0 sh2/3 gate phase-0 scores at kt>=8/12; pair-1 chunks gate
        # phase 4; q0 sh2/3 gate phases 2-3
        pend_pe = []
        for t, pair, sh in (("k", 0, 2), ("k", 0, 3), ("q", 0, 2), ("q", 0, 3),
                            ("k", 1, 0), ("k", 1, 1), ("k", 1, 2), ("k", 1, 3),
                            ("q", 1, 0), ("q", 1, 1), ("q", 1, 2), ("q", 1, 3)):
            pend_pe.append(("proj", t, pair, sh))

        # ---- attention ----
        vaug = vaugp.tile([128, NKT, HLOC, DH], BF16, tag="vaug")
        ering = eringp.tile([128, NSLOT, 512], BF16, tag="er", name="ering")

        pend_cheap = []   # non-PE (or tiny-PE) deferred stages
        pend_wo = []      # wo units
        stacks = {}
        wo_ready = set()

        def emit_psv(kt):
            # V projection for k-tile kt, all 4 heads, x^T stationary
            psv = pwop.tile([128, HLOC, DH], F32, tag="wo", name="psv")
            for c in range(NDCH):
                nc.tensor.matmul(
                    psv,
                    lhsT=xtiles[("v", c)][:, ts(kt, 128)],
                    rhs=w_sb["v"][:, c, :],
                    start=(c == 0),
                    stop=(c == NDCH - 1),
                )
            nc.vector.tensor_tensor(
                out=vaug[:, kt, :, :], in0=psv, in1=bvbc,
                op=mybir.AluOpType.add)

        def emit_scores(pair, qh, kt, gk):
            a = (2 * kt) % 4
            sa = (2 * gk) % NSLOT
            for i, lo in ((0, 0), (1, 64)):
                nc.tensor.matmul(
                    squad[:, a + i, :],
                    lhsT=kt_sb[ds(lo, 64), pair, ts(kt, 128)],
                    rhs=qt_sb[ds(lo, 64), pair, ts(qh, QW)],
                    start=True, stop=True,
                )
            nc.scalar.activation(ering[:, sa:sa + 2, :], squad[:, a:a + 2, :],
                                 mybir.ActivationFunctionType.Exp)

        def clear_bank(ps):
            # start=True clears has_written for the WHOLE bank, so banks
            # shared by concurrent tile-positioned matmuls are cleared by one
            # full-width zero matmul; the real matmuls then run start=False
            # (overwrite-where-bit-clear, accumulate-where-set).
            nc.tensor.matmul(ps, lhsT=wmt[:, 0:128], rhs=wmt,
                             start=True, stop=False, skip_group_check=True)

        def emit_ctx(pair, kt, gk, pctx):
            sa = (2 * gk) % NSLOT
            for i, lo in ((0, 0), (1, 64)):
                nc.tensor.matmul(
                    pctx[ds(lo, 64), :],
                    lhsT=vaug[:, kt, 2 * pair + i, :],
                    rhs=ering[:, sa + i, :],
                    start=False,
                    stop=(kt == NKT - 1),
                    skip_group_check=True,
                )

        def emit_z(pair, kt, gk, pz):
            # kt odd: 4 concurrent M=1 ones-matmuls for (A/B) x (kt-1, kt)
            for j in range(4):
                sa = (2 * (gk - 1 + j // 2) + j % 2) % NSLOT
                nc.tensor.matmul(
                    pz[ds(32 * j, 1), :],
                    lhsT=ones_t,
                    rhs=ering[:, sa, :],
                    start=False,
                    stop=(kt == NKT - 1),
                    tile_position=(0, 32 * j),
                    skip_group_check=True,
                )

        def norm_a(pair, qh, pctx, pz):
            # evacuate Z + ctx, then broadcast-sum (Z_even + Z_odd per head
            # across 64 partitions) via a K=128 selector matmul; non-Z rows
            # of pz were zeroed once at startup so they contribute 0
            zsb = zsbp.tile([128, 512], BF16, tag="z", name="zsb")
            nc.vector.tensor_copy(out=zsb, in_=pz)
            cxu = cxp.tile([128, 512], BF16, tag="cx", name="cxu")
            nc.vector.tensor_copy(out=cxu, in_=pctx)
            # re-arm both accumulation banks for the next phase
            clear_bank(pz)
            clear_bank(pctx)
            bc = pflp.tile([128, 512], F32, tag="fl", name="bc")
            nc.tensor.matmul(bc, lhsT=sel128, rhs=zsb, start=True, stop=True)
            pend_cheap.append(lambda: norm_b(pair, qh, cxu, bc))

        def norm_b(pair, qh, cxu, bc):
            rec = recp.tile([128, 512], F32, tag="rec", name="rec_t")
            nc.vector.reciprocal_approx_fast(out=rec, in_=bc)
            pend_cheap.append(lambda: norm_c(pair, qh, cxu, rec))

        def norm_c(pair, qh, cxu, rec):
            if qh not in stacks:
                stacks[qh] = stackp.tile([128, 2, QW], BF16, tag="stack",
                                         name="stack_t")
            nc.vector.tensor_mul(stacks[qh][:, pair, :], cxu, rec)
            if pair == 1:
                wo_ready.add(qh)
                for qt in range(QW // 128):
                    for od in range(2):
                        pend_wo.append((qh, qt, od))

        def emit_wo(qh, qt, od, wobank):
            stack_t = stacks[qh]
            pw = wobank.tile([128, 512], F32,
                             tag="wo" if wobank is pwop else "fl", name="pw")
            for t in range(2):
                nc.tensor.matmul(
                    pw,
                    lhsT=stack_t[:, t, ts(qt, 128)],
                    rhs=wo_sb[:, t, ts(od, 512)],
                    start=(t == 0), stop=(t == 1),
                )
            ob = obp.tile([128, 512], BF16, tag="ob", name="ob_t")
            nc.vector.tensor_copy(out=ob, in_=pw)
            nc.gpsimd.dma_start(
                out=outp[ds(qh * QW + qt * 128, 128), ts(od, 512)], in_=ob)

        # ---- phases ----
        gk = 0
        ctxq = []       # (pair, qh, kt, gk, pctx, pz)
        CTX_LAG = 3
        pctx_t = pctxp.tile([128, 512], F32, tag="ctx", name="pctx")
        pz_t = pzp.tile([128, 512], F32, tag="z", name="pz")
        clear_bank(pz_t)
        clear_bank(pctx_t)

        def pop_ctx():
            pair, qh, kt, g, pctx, pz = ctxq.pop(0)
            emit_ctx(pair, kt, g, pctx)
            if kt % 2 == 1:
                emit_z(pair, kt, g, pz)
            if kt == NKT - 1:
                pend_cheap.append(lambda: norm_a(pair, qh, pctx, pz))

        for phase in range(2 * NQH):
            pair, qh = phase // NQH, phase % NQH
            for kt in range(NKT):
                emit_scores(pair, qh, kt, gk)
                if phase == 0:
                    emit_psv(kt)
                ctxq.append((pair, qh, kt, gk, pctx_t, pz_t))
                gk += 1
                while len(ctxq) > CTX_LAG:
                    pop_ctx()
                while pend_cheap:
                    pend_cheap.pop(0)()
                if pend_wo and pend_wo[0][0] in wo_ready:
                    qh_, qt_, od_ = pend_wo.pop(0)
                    bank = pwop if (qt_ * 2 + od_) % 2 == 0 else pflp
                    emit_wo(qh_, qt_, od_, bank)
                elif pend_pe and kt % 2 == 0:
                    _, t, pr, sh = pend_pe.pop(0)
                    ps = pflp.tile([128, 512], F32, tag="fl", name="psD")
                    proj_chunk(t, pr, sh, ps)

        # tail: drain remaining ctx, norms, wo
        while ctxq:
            pop_ctx()
            while pend_cheap:
                pend_cheap.pop(0)()
        while pend_cheap:
            pend_cheap.pop(0)()
        i = 0
        while pend_wo:
            qh_, qt_, od_ = pend_wo.pop(0)
            bank = pwop if i % 2 == 0 else pflp
            emit_wo(qh_, qt_, od_, bank)
            i += 1
        while pend_cheap:
            pend_cheap.pop(0)()

    nc.compile()
    return nc


_NC = None


def _get_nc():
    global _NC
    if _NC is None:
        _NC = _build_program()
    return _NC


def _host_prep(query, key, value, Wq, bq, Wk, bk, Wv, bv, Wo, bo):
    bf16 = ml_dtypes.bfloat16
    f32 = np.float32
    q = np.asarray(query, f32)
    k = np.asarray(key, f32)
    v = np.asarray(value, f32)
    Wq = np.asarray(Wq, f32)
    Wk = np.asarray(Wk, f32)
    Wv = np.asarray(Wv, f32)
    Wo = np.asarray(Wo, f32)
    bq = np.asarray(bq, f32)
    bk = np.asarray(bk, f32)
    bv = np.asarray(bv, f32)

    scale = np.float32(1.0 / np.sqrt(DH))
    xqT = np.ascontiguousarray(q.transpose(0, 2, 1)).astype(bf16)
    xkT = np.ascontiguousarray(k.transpose(0, 2, 1)).astype(bf16)
    xvT = np.ascontiguousarray(v.transpose(0, 2, 1)).astype(bf16)

    in_maps = []
    for c in range(NCORES):
        b = c // 4
        g = c % 4
        sl = slice(g * DLOC, (g + 1) * DLOC)
        in_maps.append({
            "xqT": xqT[b], "xkT": xkT[b], "xvT": xvT[b],
            "wq": np.ascontiguousarray(Wq[:, sl] * scale).astype(bf16),
            "wk": np.ascontiguousarray(Wk[:, sl]).astype(bf16),
            "wv": np.ascontiguousarray(Wv[:, sl]).astype(bf16),
            "bq": np.ascontiguousarray((bq[sl] * scale).reshape(DLOC, 1)),
            "bk": np.ascontiguousarray(bk[sl].reshape(DLOC, 1)),
            "bv": np.ascontiguousarray(bv[sl].reshape(1, DLOC)),
            "wo": np.ascontiguousarray(Wo[sl, :]).astype(bf16),
        })
    return in_maps


def _run(in_maps, trace=False):
    nc = _get_nc()
    return run_bass_kernel_spmd(nc, in_maps, list(range(NCORES)), trace=trace)


def kernel(query, key, value, Wq, bq, Wk, bk, Wv, bv, Wo, bo):
    in_maps = _host_prep(query, key, value, Wq, bq, Wk, bk, Wv, bv, Wo, bo)
    res = _run(in_maps)
    out = np.zeros((B, N, D), np.float32)
    for c in range(NCORES):
        out[c // 4] += np.asarray(res.results[c]["outp"], np.float32)
    out += np.asarray(bo, np.float32)[None, None, :]
    return out


# revision 14
# speedup vs baseline: 1.0017x; 1.0017x over previous
"""Multi-head attention (B=2, N=2048, D=1024, H=16, dh=64) on 8 TRN2 cores.

Sharding: (batch x head-group) -- core c handles batch c//4 and heads
[4*(c%4), 4*(c%4)+4) (256 local dims = 2 head-pairs). Host sums 4
partials per batch and adds bo.

Per-core pipeline (v2): the PE array is time-packed via tile_position
so the attention matmuls use the full 128x128 array:
  - scores: heads 2t/2t+1 run CONCURRENTLY as 64x128 row-tiles
    (contraction dh=64; lhsT/rhs at partitions 0-63 vs 64-127).
  - ctx: the same pair runs concurrently as 128x64 col-tiles
    (lhsT = V_h [128,64]; outputs at psum partitions 0-63 / 64-127).
  - softmax denominator Z: M=1 ones-matmuls at col positions
    (0,0),(0,32),(0,64),(0,96) -- 4 concurrent streams covering
    (head A/B) x (even/odd kt), accumulated over kt in ONE psum bank.
    A K=4 selector matmul later broadcasts (Z_even+Z_odd) per head
    across 64 partitions.
  - exp runs on ACT as [128,1024] insts reading a 4-slot psum quad
    (2 slots per kt: A|B), ring-recycled; ACT is the steady-state
    bottleneck at ~1147ns/kt.
Work is organized as 8 phases (pair-major, q-half 512): pair p, then
q-halves 0..3. V projection (x^T stationary) rides phase 0-1; Q/K
projections for pair 1 ride phases 1-3 as deferred PE units; the
output projection (per q-half, both pairs) rides phases 4-7.
"""

import numpy as np
import ml_dtypes
from contextlib import ExitStack

import concourse.bass as bass
import concourse.tile as tile
from concourse import bacc, mybir
from concourse.bass import ts, ds
from concourse.bass_utils import run_bass_kernel_spmd

BF16 = mybir.dt.bfloat16
F32 = mybir.dt.float32

B = 2
N = 2048          # tokens per batch
D = 1024          # model dim
NCORES = 8
HLOC = 4          # heads per core
DLOC = 256        # local dims per core
DH = 64
NKT = N // 128    # 16 k-tiles
NDCH = D // 128   # 8 d-chunks
QW = 512          # q-half width
NQH = N // QW     # 4 q-halves
NSLOT = 40        # e-ring slots of [128, 512]


def _build_program():
    nc = bacc.Bacc("TRN2", target_bir_lowering=False, debug=True)

    xT = {}
    w = {}
    for t in ("q", "k", "v"):
        xT[t] = nc.dram_tensor(f"x{t}T", [D, N], BF16, kind="ExternalInput").ap()
        w[t] = nc.dram_tensor(f"w{t}", [D, DLOC], BF16, kind="ExternalInput").ap()
    bias = {}
    for t in ("q", "k"):
        bias[t] = nc.dram_tensor(f"b{t}", [DLOC, 1], F32, kind="ExternalInput").ap()
    bvd = nc.dram_tensor("bv", [1, DLOC], F32, kind="ExternalInput").ap()
    wo = nc.dram_tensor("wo", [DLOC, D], BF16, kind="ExternalInput").ap()
    outp = nc.dram_tensor("outp", [N, D], BF16, kind="ExternalOutput").ap()

    with ExitStack() as ctx:
        tc = ctx.enter_context(tile.TileContext(nc))

        const = ctx.enter_context(tc.tile_pool(name="const", bufs=1))
        xqkp = ctx.enter_context(tc.tile_pool(name="xqk", bufs=32))
        xvp = ctx.enter_context(tc.tile_pool(name="xv", bufs=8))
        qkp = ctx.enter_context(tc.tile_pool(name="qk", bufs=1))
        vaugp = ctx.enter_context(tc.tile_pool(name="vaug", bufs=1))
        eringp = ctx.enter_context(tc.tile_pool(name="ering", bufs=1))
        cxp = ctx.enter_context(tc.tile_pool(name="cxu", bufs=2))
        zsbp = ctx.enter_context(tc.tile_pool(name="zsb", bufs=2))
        recp = ctx.enter_context(tc.tile_pool(name="rec", bufs=2))
        stackp = ctx.enter_context(tc.tile_pool(name="stack", bufs=4))
        obp = ctx.enter_context(tc.tile_pool(name="ob", bufs=4))

        # PSUM: squad 4 banks (scores ring), pctx 1 (ctx accum), pz 1
        # (Z accum), pwo 1 (psv then wo), pfl 1 (proj chunks, bc, wo)
        psqp = ctx.enter_context(tc.tile_pool(name="psq", bufs=1, space="PSUM"))
        pctxp = ctx.enter_context(tc.tile_pool(name="pctx", bufs=1, space="PSUM"))
        pzp = ctx.enter_context(tc.tile_pool(name="pz", bufs=1, space="PSUM"))
        pwop = ctx.enter_context(tc.tile_pool(name="pwo", bufs=1, space="PSUM"))
        pflp = ctx.enter_context(tc.tile_pool(name="pfl", bufs=1, space="PSUM"))

        # ---- constants + x loads, ordered for just-in-time arrival ----
        w_sb = {}
        b_sb = {}
        xtiles = {}

        def load_x(t, hf):
            for c in range(NDCH):
                xt_ = xqkp.tile([128, 1024], BF16, tag="x", name="xt")
                nc.sync.dma_start(out=xt_, in_=xT[t][ts(c, 128), ts(hf, 1024)])
                xtiles[(t, c, hf)] = xt_

        for t in ("q", "k"):
            w_sb[t] = const.tile([128, NDCH, DLOC], BF16, tag=f"w{t}", name=f"w{t}sb")
            nc.sync.dma_start(out=w_sb[t], in_=w[t].rearrange("(c p) m -> p c m", p=128))
            b_sb[t] = const.tile([128, 2, 1], F32, tag=f"b{t}", name=f"b{t}sb")
            nc.sync.dma_start(out=b_sb[t],
                              in_=bias[t].rearrange("(t p) o -> p t o", p=128))
        load_x("k", 0)
        load_x("q", 0)
        load_x("k", 1)
        load_x("q", 1)
        w_sb["v"] = const.tile([128, NDCH, DLOC], BF16, tag="wv", name="wvsb")
        nc.sync.dma_start(out=w_sb["v"], in_=w["v"].rearrange("(c p) m -> p c m", p=128))
        for c in range(NDCH):
            xt_ = xvp.tile([128, N], BF16, tag="xv", name="xvt")
            nc.sync.dma_start(out=xt_, in_=xT["v"][ts(c, 128), :])
            xtiles[("v", c)] = xt_
        bvbc = const.tile([128, HLOC, DH], F32, tag="bvbc")
        seg = bvd[0, :]
        nc.sync.dma_start(
            out=bvbc,
            in_=bass.AP(tensor=seg.tensor, offset=seg.offset,
                        ap=[[0, 128]] + list(seg.ap)))
        wo_sb = const.tile([128, 2, D], BF16, tag="wo")
        nc.sync.dma_start(out=wo_sb, in_=wo.rearrange("(t p) d -> p t d", p=128))

        # warm the exp table load off the critical path
        warm = const.tile([128, 1], BF16, tag="warm")
        nc.scalar.activation(warm, b_sb["q"][:, 0, :], mybir.ActivationFunctionType.Exp)

        # ones column for Z matmuls; K=4 selector for the Z broadcast:
        # bc[m] = sum_j sel4[j, m] * zsb[j]; rows 0-63 <- z[0]+z[2] (head A
        # even+odd kt), rows 64-127 <- z[1]+z[3] (head B)
        ones_t = const.tile([128, 1], BF16, tag="ones")
        nc.vector.memset(ones_t, 1.0)
        sel128 = const.tile([128, 128], BF16, tag="sel128")
        nc.vector.memset(sel128, 0.0)
        nc.vector.memset(sel128[0:1, 0:64], 1.0)
        nc.vector.memset(sel128[64:65, 0:64], 1.0)
        nc.vector.memset(sel128[32:33, 64:128], 1.0)
        nc.vector.memset(sel128[96:97, 64:128], 1.0)

        # ---- PE warm-up burst during the DMA lead-in (HAM clock gate) ----
        wmt = const.tile([128, 512], BF16, tag="wmt")
        nc.vector.memset(wmt, 0.0)
        squad = psqp.tile([128, 4, 512], F32, tag="sq", name="squad")
        for i in range(10):
            nc.tensor.matmul(squad[:, 0, :], lhsT=wmt[:, 0:128], rhs=wmt,
                             start=(i == 0), stop=(i == 9))

        # ---- projections ----
        # Q^T/K^T [dl, tok] per pair (dl-tile), W stationary. One chunk =
        # one (t, pair, sh) with 8 accumulating c-matmuls into a [128,512]
        # psum + a bias-add evac. Prelude: K/Q pair0 sh0 on squad slots;
        # the rest ride phases as deferred PE units.
        qt_sb = qkp.tile([128, 2, N], BF16, tag="qt")
        kt_sb = qkp.tile([128, 2, N], BF16, tag="kt")
        dest = {"q": qt_sb, "k": kt_sb}

        def proj_chunk(t, pair, sh, ps):
            for c in range(NDCH):
                nc.tensor.matmul(
                    ps,
                    lhsT=w_sb[t][:, c, ds(pair * 128, 128)],
                    rhs=xtiles[(t, c, sh // 2)][:, ds((sh % 2) * 512, 512)],
                    start=(c == 0),
                    stop=(c == NDCH - 1),
                )
            nc.vector.tensor_scalar_add(
                dest[t][:, pair, ts(sh, 512)], ps, b_sb[t][:, pair, :])

        proj_chunk("k", 0, 0, squad[:, 0, :])
        proj_chunk("q", 0, 0, squad[:, 1, :])
        proj_chunk("k", 0, 1, squad[:, 2, :])
        proj_chunk("q", 0, 1, squad[:, 3, :])

        # deferred PE units (each ~2us of PE work), ordered by need:
        # k0 sh2/3 gate phase-0 scores at kt>=8/12; pair-1 chunks gate
        # phase 4; q0 sh2/3 gate phases 2-3
        pend_pe = []
        for t, pair, sh in (("k", 0, 2), ("k", 0, 3), ("q", 0, 2), ("q", 0, 3),
                            ("k", 1, 0), ("k", 1, 1), ("k", 1, 2), ("k", 1, 3),
                            ("q", 1, 0), ("q", 1, 1), ("q", 1, 2), ("q", 1, 3)):
            pend_pe.append(("proj", t, pair, sh))

        # ---- attention ----
        vaug = vaugp.tile([128, NKT, HLOC, DH], BF16, tag="vaug")
        ering = eringp.tile([128, NSLOT, 512], BF16, tag="er", name="ering")

        pend_cheap = []   # non-PE (or tiny-PE) deferred stages
        pend_wo = []      # wo units
        stacks = {}
        wo_ready = set()

        def emit_psv(kt):
            # V projection for k-tile kt, all 4 heads, x^T stationary
            psv = pwop.tile([128, HLOC, DH], F32, tag="wo", name="psv")
            for c in range(NDCH):
                nc.tensor.matmul(
                    psv,
                    lhsT=xtiles[("v", c)][:, ts(kt, 128)],
                    rhs=w_sb["v"][:, c, :],
                    start=(c == 0),
                    stop=(c == NDCH - 1),
                )
            nc.vector.tensor_tensor(
                out=vaug[:, kt, :, :], in0=psv, in1=bvbc,
                op=mybir.AluOpType.add)

        def emit_scores(pair, qh, kt, gk):
            a = (2 * kt) % 4
            sa = (2 * gk) % NSLOT
            for i, lo in ((0, 0), (1, 64)):
                nc.tensor.matmul(
                    squad[:, a + i, :],
                    lhsT=kt_sb[ds(lo, 64), pair, ts(kt, 128)],
                    rhs=qt_sb[ds(lo, 64), pair, ts(qh, QW)],
                    start=True, stop=True,
                )
            nc.scalar.activation(ering[:, sa:sa + 2, :], squad[:, a:a + 2, :],
                                 mybir.ActivationFunctionType.Exp)

        def clear_bank(ps):
            # start=True clears has_written for the WHOLE bank, so banks
            # shared by concurrent tile-positioned matmuls are cleared by one
            # full-width zero matmul; the real matmuls then run start=False
            # (overwrite-where-bit-clear, accumulate-where-set).
            nc.tensor.matmul(ps, lhsT=wmt[:, 0:128], rhs=wmt,
                             start=True, stop=False, skip_group_check=True)

        def emit_ctx(pair, kt, gk, pctx):
            sa = (2 * gk) % NSLOT
            for i, lo in ((0, 0), (1, 64)):
                nc.tensor.matmul(
                    pctx[ds(lo, 64), :],
                    lhsT=vaug[:, kt, 2 * pair + i, :],
                    rhs=ering[:, sa + i, :],
                    start=False,
                    stop=(kt == NKT - 1),
                    skip_group_check=True,
                )

        def emit_z(pair, kt, gk, pz):
            # kt odd: 4 concurrent M=1 ones-matmuls for (A/B) x (kt-1, kt)
            for j in range(4):
                sa = (2 * (gk - 1 + j // 2) + j % 2) % NSLOT
                nc.tensor.matmul(
                    pz[ds(32 * j, 1), :],
                    lhsT=ones_t,
                    rhs=ering[:, sa, :],
                    start=False,
                    stop=(kt == NKT - 1),
                    tile_position=(0, 32 * j),
                    skip_group_check=True,
                )

        def norm_a(pair, qh, pctx, pz):
            # evacuate Z + ctx, then broadcast-sum (Z_even + Z_odd per head
            # across 64 partitions) via a K=128 selector matmul; non-Z rows
            # of pz were zeroed once at startup so they contribute 0
            zsb = zsbp.tile([128, 512], BF16, tag="z", name="zsb")
            nc.vector.tensor_copy(out=zsb, in_=pz)
            cxu = cxp.tile([128, 512], BF16, tag="cx", name="cxu")
            nc.vector.tensor_copy(out=cxu, in_=pctx)
            # re-arm both accumulation banks for the next phase
            clear_bank(pz)
            clear_bank(pctx)
            bc = pflp.tile([128, 512], F32, tag="fl", name="bc")
            nc.tensor.matmul(bc, lhsT=sel128, rhs=zsb, start=True, stop=True)
            pend_cheap.append(lambda: norm_b(pair, qh, cxu, bc))

        def norm_b(pair, qh, cxu, bc):
            rec = recp.tile([128, 512], F32, tag="rec", name="rec_t")
            nc.vector.reciprocal_approx_fast(out=rec, in_=bc)
            pend_cheap.append(lambda: norm_c(pair, qh, cxu, rec))

        def norm_c(pair, qh, cxu, rec):
            if qh not in stacks:
                stacks[qh] = stackp.tile([128, 2, QW], BF16, tag="stack",
                                         name="stack_t")
            nc.vector.tensor_mul(stacks[qh][:, pair, :], cxu, rec)
            if pair == 1:
                wo_ready.add(qh)
                for qt in range(QW // 128):
                    for od in range(2):
                        pend_wo.append((qh, qt, od))

        def emit_wo(qh, qt, od, wobank):
            stack_t = stacks[qh]
            pw = wobank.tile([128, 512], F32,
                             tag="wo" if wobank is pwop else "fl", name="pw")
            for t in range(2):
                nc.tensor.matmul(
                    pw,
                    lhsT=stack_t[:, t, ts(qt, 128)],
                    rhs=wo_sb[:, t, ts(od, 512)],
                    start=(t == 0), stop=(t == 1),
                )
            ob = obp.tile([128, 512], BF16, tag="ob", name="ob_t")
            nc.vector.tensor_copy(out=ob, in_=pw)
            nc.gpsimd.dma_start(
                out=outp[ds(qh * QW + qt * 128, 128), ts(od, 512)], in_=ob)

        # ---- phases ----
        gk = 0
        ctxq = []       # (pair, qh, kt, gk, pctx, pz)
        CTX_LAG = 3
        pctx_t = pctxp.tile([128, 512], F32, tag="ctx", name="pctx")
        pz_t = pzp.tile([128, 512], F32, tag="z", name="pz")
        clear_bank(pz_t)
        clear_bank(pctx_t)

        def pop_ctx():
            pair, qh, kt, g, pctx, pz = ctxq.pop(0)
            emit_ctx(pair, kt, g, pctx)
            if kt % 2 == 1:
                emit_z(pair, kt, g, pz)
            if kt == NKT - 1:
                pend_cheap.append(lambda: norm_a(pair, qh, pctx, pz))

        for phase in range(2 * NQH):
            pair, qh = phase // NQH, phase % NQH
            for kt in range(NKT):
                emit_scores(pair, qh, kt, gk)
                if phase == 0:
                    emit_psv(kt)
                ctxq.append((pair, qh, kt, gk, pctx_t, pz_t))
                gk += 1
                while len(ctxq) > CTX_LAG:
                    pop_ctx()
                while pend_cheap:
                    pend_cheap.pop(0)()
                if pend_wo and pend_wo[0][0] in wo_ready:
                    qh_, qt_, od_ = pend_wo.pop(0)
                    bank = pwop if (qt_ * 2 + od_) % 2 == 0 else pflp
                    emit_wo(qh_, qt_, od_, bank)
                elif pend_pe and kt % 2 == 0:
                    _, t, pr, sh = pend_pe.pop(0)
                    ps = pflp.tile([128, 512], F32, tag="fl", name="psD")
                    proj_chunk(t, pr, sh, ps)

        # tail: drain remaining ctx, norms, wo
        while ctxq:
            pop_ctx()
            while pend_cheap:
                pend_cheap.pop(0)()
        while pend_cheap:
            pend_cheap.pop(0)()
        i = 0
        while pend_wo:
            qh_, qt_, od_ = pend_wo.pop(0)
            bank = pwop if i % 2 == 0 else pflp
            emit_wo(qh_, qt_, od_, bank)
            i += 1
        while pend_cheap:
            pend_cheap.pop(0)()

    nc.compile()
    return nc


_NC = None


def _get_nc():
    global _NC
    if _NC is None:
        _NC = _build_program()
    return _NC


def _host_prep(query, key, value, Wq, bq, Wk, bk, Wv, bv, Wo, bo):
    bf16 = ml_dtypes.bfloat16
    f32 = np.float32
    q = np.asarray(query, f32)
    k = np.asarray(key, f32)
    v = np.asarray(value, f32)
    Wq = np.asarray(Wq, f32)
    Wk = np.asarray(Wk, f32)
    Wv = np.asarray(Wv, f32)
    Wo = np.asarray(Wo, f32)
    bq = np.asarray(bq, f32)
    bk = np.asarray(bk, f32)
    bv = np.asarray(bv, f32)

    scale = np.float32(1.0 / np.sqrt(DH))
    xqT = np.ascontiguousarray(q.transpose(0, 2, 1)).astype(bf16)
    xkT = np.ascontiguousarray(k.transpose(0, 2, 1)).astype(bf16)
    xvT = np.ascontiguousarray(v.transpose(0, 2, 1)).astype(bf16)

    in_maps = []
    for c in range(NCORES):
        b = c // 4
        g = c % 4
        sl = slice(g * DLOC, (g + 1) * DLOC)
        in_maps.append({
            "xqT": xqT[b], "xkT": xkT[b], "xvT": xvT[b],
            "wq": np.ascontiguousarray(Wq[:, sl] * scale).astype(bf16),
            "wk": np.ascontiguousarray(Wk[:, sl]).astype(bf16),
            "wv": np.ascontiguousarray(Wv[:, sl]).astype(bf16),
            "bq": np.ascontiguousarray((bq[sl] * scale).reshape(DLOC, 1)),
            "bk": np.ascontiguousarray(bk[sl].reshape(DLOC, 1)),
            "bv": np.ascontiguousarray(bv[sl].reshape(1, DLOC)),
            "wo": np.ascontiguousarray(Wo[sl, :]).astype(bf16),
        })
    return in_maps


def _run(in_maps, trace=False):
    nc = _get_nc()
    return run_bass_kernel_spmd(nc, in_maps, list(range(NCORES)), trace=trace)


def kernel(query, key, value, Wq, bq, Wk, bk, Wv, bv, Wo, bo):
    in_maps = _host_prep(query, key, value, Wq, bq, Wk, bk, Wv, bv, Wo, bo)
    res = _run(in_maps)
    out = np.zeros((B, N, D), np.float32)
    for c in range(NCORES):
        out[c // 4] += np.asarray(res.results[c]["outp"], np.float32)
    out += np.asarray(bo, np.float32)[None, None, :]
    return out
